# revision 6
# baseline (speedup 1.0000x reference)
"""Blended-MoE 3-layer MLP (moe_routing) on 8 trn2 NeuronCores.

Math: per layer  z[b,o] = sum_e blend[e,b] * (w[e] @ h[b] + bias[e])[o],
ELU between layers.  Single contraction per layer:

    z[b,o] = sum_{(e,i)} (blend[e,b] * hT[i,b]) * wT[(e,i), o]
           + sum_e blend[e,b] * bias[e,o]          (bias via one K=8 matmul)

Data-parallel across 8 cores (128 batch rows each); expert weights are
replicated, host-side pre-transposed into SBUF-image layout.

The kernel is HBM-bound: 10.55 MB of fp16 weights per core at ~390 GB/s.
Schedule notes:
  - weights ride the sync HW-DGE queue in strict consumption order; the
    small inputs (xt+identity, blend broadcast, biases) ride the gpsimd
    SWDGE queue in parallel so both queues initialize concurrently and
    weight bytes start as early as possible
  - layer 0 uses true K=480 (three full 128-row k-tiles + one 96-row
    partial per expert) instead of zero-padding to 512
  - the blend broadcast is only [128, E*128] (the i-tile dim is redundant);
    the he expansion runs per (e, i-tile) on DVE
  - weight-group granularity is the matmul batching unit: each group's
    matmuls wait on one DMA sem, so the PE runs in long bursts (p-state)
  - layer 2 is split into two output-column halves (160+151) so the first
    half's PSUM->SBUF copy + output DMA overlap the second half's matmuls
"""

import numpy as np

import concourse.bass as bass
import concourse.mybir as mybir
import concourse.tile as tile
from concourse.bass_utils import run_bass_kernel_spmd

import bass_rust

# ---- config ----------------------------------------------------------------
N_CORES = 8
B, E = 1024, 8
DIN, D1, D2, D3 = 480, 512, 512, 311
O2A = 160                       # layer-2 output split: [0:160), [160:311)
O2B = D3 - O2A

PROFILE = {"trace": False, "tmpdir": None}
LAST_RESULT = [None]

_NC_CACHE = {}
_SPLIT_N = [0]

# weight-group tiling (number of k-tiles per DMA transfer)
W0A_GROUPS = [12, 12]           # e-major over (e, it<3), 24 full k-tiles
W1_GROUPS = [16, 16]            # half-major (half, e, sub), 32 k-tiles
W2_GROUPS = [16, 12, 4]         # last group kept small for a short tail


def _locate(groups, j):
    """k-tile j -> (group index, offset within group)."""
    g = 0
    while j >= groups[g]:
        j -= groups[g]
        g += 1
    return g, j


def _order1():
    """(e, it) consumption order for layers 1/2: it-half-major."""
    return [
        (e, 2 * half + sub)
        for half in range(2)
        for e in range(E)
        for sub in range(2)
    ]


def _split_multi_waits(nc, max_waits=1):
    """This container's walrus only supports one sync-wait command per
    instruction; spill extras onto same-engine NOPs inserted just before."""
    for f in nc.m.functions:
        for bb in f.blocks:
            insts = bb.instructions
            i = 0
            while i < len(insts):
                inst = insts[i]
                si = inst.sync_info
                if si is not None and len(si.on_wait) > max_waits:
                    waits = list(si.on_wait)
                    extra, keep = waits[:-max_waits], waits[-max_waits:]
                    for w in extra:
                        _SPLIT_N[0] += 1
                        nop = mybir.InstNoOp(
                            name=f"wsplit-{_SPLIT_N[0]}", ins=[], outs=[]
                        )
                        nop.engine = inst.engine
                        nop.sync_info = bass_rust.SyncInfo(
                            on_wait=[w], on_update=[]
                        )
                        insts.insert(i, nop)
                        i += 1
                    inst.sync_info = bass_rust.SyncInfo(
                        on_wait=keep, on_update=list(si.on_update)
                    )
                i += 1


class _FastTailTC(tile.TileContext):
    """Tile's kernel tail is drain-with-per-sem-waits + 2 all-engine
    barriers + per-sem clears; the per-sem waits explode into ~70 NOPs per
    engine under the single-wait walrus (~8 us).  All DMAs except the
    output writeback have already been observed by their consumers, so a
    barrier (engines idle, all triggers issued) followed by the range-based
    DMA drain + semaphore clear inside clear_and_free_semaphores is enough."""

    def _drain_and_barrier(self, tick_clock, wait_clock):
        nc = self.nc
        nc.all_engine_barrier()
        popped = nc._tile_sem_poison_stack.pop()
        assert popped is self._sem_poison
        assert self.sems is not None
        nc.clear_and_free_semaphores(list(self.sems.allocated().values()))


def _build_nc():
    f32 = mybir.dt.float32
    dt = mybir.dt.float16
    nc = bass.Bass()

    # ---- DRAM tensors ----
    # xti: [xt image (it,b) | 128x128 identity]
    xti_d = nc.dram_tensor("xti", [128, 512 + 128], dt, kind="ExternalInput")
    # bbsm[p, e*128 + b] = blend[e, b]  (partition-broadcast)
    bbsm_d = nc.dram_tensor("bbsm", [128, E * 128], dt, kind="ExternalInput")
    # small: [ blT (8x128) | bias0 (8x512) | bias1 (8x512) | bias2 (8x311) ]
    SMALL_COLS = 128 + D1 + D2 + D3
    small_d = nc.dram_tensor("small", [8, SMALL_COLS], dt, kind="ExternalInput")

    w0a_d = nc.dram_tensor("w0a", [128, 24 * D1], dt, kind="ExternalInput")
    w0b_d = nc.dram_tensor("w0b", [96, 8 * D1], dt, kind="ExternalInput")
    w1_d = nc.dram_tensor("w1", [128, 32 * D2], dt, kind="ExternalInput")
    w2_d = nc.dram_tensor("w2", [128, 32 * D3], dt, kind="ExternalInput")
    out_d = nc.dram_tensor("out", [128, D3], f32, kind="ExternalOutput")

    with _FastTailTC(nc) as tc:
        with (
            tc.tile_pool(name="const", bufs=1) as const,
            tc.tile_pool(name="w", bufs=16) as wpool,
            tc.tile_pool(name="acts", bufs=2) as acts,
            tc.tile_pool(name="tmp", bufs=2) as tmp,
            tc.tile_pool(name="zp", bufs=2, space="PSUM") as zp,
            tc.tile_pool(name="zo", bufs=1, space="PSUM") as zo,
            tc.tile_pool(name="tp", bufs=2, space="PSUM") as tp,
        ):
            # ---- inputs lead the sync HWDGE stream (SWDGE starts too late
            # and its Q7 launches contend with the SDMA engines) ----
            xti_sb = const.tile([128, 512 + 128], dt)
            nc.sync.dma_start(xti_sb[:], xti_d[:])
            bbsm = const.tile([128, E * 128], dt)
            nc.sync.dma_start(bbsm[:], bbsm_d[:])
            small_sb = const.tile([8, SMALL_COLS], dt)
            nc.sync.dma_start(small_sb[:], small_d[:])

            ident = xti_sb[:, 512:640]
            bl8 = small_sb[:, 0:128]
            waug = []
            off = 128
            for n in (D1, D2, D3):
                waug.append(small_sb[:, off : off + n])
                off += n

            # ---- weight stream: strict consumption order on sync HWDGE ----
            w0a = []
            c = 0
            for g, nt in enumerate(W0A_GROUPS):
                t = wpool.tile([128, nt * D1], dt, tag=f"w0a{g}", bufs=1)
                nc.sync.dma_start(t[:], w0a_d[:, c * D1 : (c + nt) * D1])
                w0a.append(t)
                c += nt
            w0b = wpool.tile([96, 8 * D1], dt, tag="w0b", bufs=1)
            nc.sync.dma_start(w0b[:], w0b_d[:])
            w1 = []
            c = 0
            for g, nt in enumerate(W1_GROUPS):
                t = wpool.tile([128, nt * D2], dt, tag=f"w1{g}", bufs=1)
                nc.sync.dma_start(t[:], w1_d[:, c * D2 : (c + nt) * D2])
                w1.append(t)
                c += nt
            w2 = []
            c = 0
            for g, nt in enumerate(W2_GROUPS):
                t = wpool.tile([128, nt * D3], dt, tag=f"w2{g}", bufs=1)
                nc.sync.dma_start(t[:], w2_d[:, c * D3 : (c + nt) * D3])
                w2.append(t)
                c += nt

            def expand(he, src_ap, e, it, rows=128):
                # he[:, e*512 + it*128 + b] = src * blend[e]
                col = e * 512 + it * 128
                nc.vector.tensor_tensor(
                    he[:rows, col : col + 128],
                    src_ap,
                    bbsm[:rows, e * 128 : (e + 1) * 128],
                    mybir.AluOpType.mult,
                )

            # ---- he for layer 0 ----
            he = acts.tile([128, E * 512], dt, tag="he")
            for e in range(E):
                for it in range(3):
                    expand(he, xti_sb[:, it * 128 : (it + 1) * 128], e, it)
                expand(he, xti_sb[:96, 384:512], e, 3, rows=96)

            # ---- layer 0 (bias is the final accumulation: PE gap filler) ----
            z0 = zp.tile([128, D1], f32, tag="z")
            j = 0
            for g, t in enumerate(w0a):
                for loc in range(W0A_GROUPS[g]):
                    e, it = divmod(j, 3)
                    nc.tensor.matmul(
                        z0[:],
                        he[:, e * 512 + it * 128 : e * 512 + (it + 1) * 128],
                        t[:, loc * D1 : (loc + 1) * D1],
                        start=(j == 0),
                        stop=False,
                    )
                    j += 1
            for e in range(E):
                nc.tensor.matmul(
                    z0[:],
                    he[:96, e * 512 + 384 : e * 512 + 512],
                    w0b[:96, e * D1 : (e + 1) * D1],
                    start=False,
                    stop=False,
                )
            nc.tensor.matmul(z0[:], bl8, waug[0], start=False, stop=True)

            # ---- boundary + layers 1/2 ----
            def boundary(z, he_next, tag):
                """ELU + transpose + expansion, one 256-col half at a time.
                Returns list of per-half 'ready' hT tiles (consumed inline)."""
                for half in range(2):
                    lo, hi = half * 256, (half + 1) * 256
                    m = tmp.tile([128, 256], f32, tag=f"{tag}m{half}")
                    nc.vector.tensor_scalar(
                        m[:], z[:, lo:hi], 0.0, None, mybir.AluOpType.min
                    )
                    ex = tmp.tile([128, 256], f32, tag=f"{tag}e{half}")
                    nc.scalar.activation(
                        ex[:], m[:], mybir.ActivationFunctionType.Exp
                    )
                    p = tmp.tile([128, 256], f32, tag=f"{tag}p{half}")
                    nc.vector.tensor_scalar(
                        p[:], z[:, lo:hi], 0.0, -1.0,
                        mybir.AluOpType.max, mybir.AluOpType.add,
                    )
                    hh = tmp.tile([128, 256], dt, tag=f"{tag}h{half}")
                    nc.vector.tensor_tensor(
                        hh[:], p[:], ex[:], mybir.AluOpType.add
                    )
                    tps = tp.tile([128, 256], dt, tag=f"t{half}", bufs=1)
                    for a in range(2):
                        nc.tensor.transpose(
                            tps[:, a * 128 : (a + 1) * 128],
                            hh[:, a * 128 : (a + 1) * 128],
                            ident,
                        )
                    hT = tmp.tile([128, 256], dt, tag=f"{tag}T{half}")
                    nc.scalar.copy(hT[:], tps[:])
                    for e in range(E):
                        for s in range(2):
                            expand(
                                he_next,
                                hT[:, s * 128 : (s + 1) * 128],
                                e,
                                2 * half + s,
                            )
                    yield half

            # layer 1
            he1 = acts.tile([128, E * 512], dt, tag="he")
            z1 = zp.tile([128, D2], f32, tag="z")
            nc.tensor.matmul(z1[:], bl8, waug[1], start=True, stop=False)
            order = _order1()
            bgen = boundary(z0, he1, "b0")
            next(bgen)  # half 0 ready (queued)
            for j, (e, it) in enumerate(order):
                if j == 16:
                    next(bgen)  # queue half-1 chain after half-0 matmuls
                g, loc = _locate(W1_GROUPS, j)
                nc.tensor.matmul(
                    z1[:],
                    he1[:, e * 512 + it * 128 : e * 512 + (it + 1) * 128],
                    w1[g][:, loc * D2 : (loc + 1) * D2],
                    start=False,
                    stop=(j == 31),
                )

            # layer 2
            he2 = acts.tile([128, E * 512], dt, tag="he")
            z2 = zo.tile([128, D3], f32, tag="z2")
            nc.tensor.matmul(z2[:], bl8, waug[2], start=True, stop=False)
            bgen = boundary(z1, he2, "b1")
            next(bgen)
            for j, (e, it) in enumerate(order):
                if j == 16:
                    next(bgen)
                g, loc = _locate(W2_GROUPS, j)
                nc.tensor.matmul(
                    z2[:],
                    he2[:, e * 512 + it * 128 : e * 512 + (it + 1) * 128],
                    w2[g][:, loc * D3 : (loc + 1) * D3],
                    start=False,
                    stop=(j == 31),
                )
            out_sb = tmp.tile([128, D3], f32, tag="osb")
            nc.vector.tensor_copy(out_sb[:], z2[:])
            nc.sync.dma_start(out_d[:], out_sb[:])

    _split_multi_waits(nc)
    return nc


# ---- host-side packing -----------------------------------------------------


def _wimgs(w0, w1, w2, np_dt):
    wt = [np.ascontiguousarray(w.transpose(0, 2, 1)) for w in (w0, w1, w2)]

    # layer 0: e-major over (e, it<3) full tiles + (e, it=3) 96-row partials
    w0a = np.empty((128, 24 * D1), np.float32)
    j = 0
    for e in range(E):
        for it in range(3):
            w0a[:, j * D1 : (j + 1) * D1] = wt[0][e][it * 128 : (it + 1) * 128]
            j += 1
    w0b = np.empty((96, 8 * D1), np.float32)
    for e in range(E):
        w0b[:, e * D1 : (e + 1) * D1] = wt[0][e][384:480]

    order = _order1()
    w1i = np.empty((128, 32 * D2), np.float32)
    for j, (e, it) in enumerate(order):
        w1i[:, j * D2 : (j + 1) * D2] = wt[1][e][it * 128 : (it + 1) * 128]
    w2i = np.empty((128, 32 * D3), np.float32)
    for j, (e, it) in enumerate(order):
        w2i[:, j * D3 : (j + 1) * D3] = wt[2][e][it * 128 : (it + 1) * 128]
    return {
        "w0a": np.ascontiguousarray(w0a).astype(np_dt),
        "w0b": np.ascontiguousarray(w0b).astype(np_dt),
        "w1": np.ascontiguousarray(w1i).astype(np_dt),
        "w2": np.ascontiguousarray(w2i).astype(np_dt),
    }


def kernel(x, weight_blend, w0, b0, w1, b1, w2, b2):
    np_dt = np.float16

    if "nc" not in _NC_CACHE:
        _NC_CACHE["nc"] = _build_nc()
    nc = _NC_CACHE["nc"]

    x = np.asarray(x, np.float32)
    weight_blend = np.asarray(weight_blend, np.float32)
    wimgs = _wimgs(np.asarray(w0), np.asarray(w1), np.asarray(w2), np_dt)
    biases = [np.asarray(b, np.float32) for b in (b0, b1, b2)]
    eye = np.eye(128, dtype=np.float32)

    bc = B // N_CORES
    in_maps = []
    for c in range(N_CORES):
        sl = slice(c * bc, (c + 1) * bc)
        xT = np.zeros((4 * 128, bc), np.float32)
        xT[:DIN] = x[sl].T
        xt_img = xT.reshape(4, 128, bc).transpose(1, 0, 2).reshape(128, 4 * bc)
        xti = np.concatenate([xt_img, eye], axis=1)
        bl = weight_blend[:, sl]  # (8, 128)
        small_img = np.concatenate([bl] + biases, axis=1)
        bbsm = np.broadcast_to(bl[None, :, :], (128, E, bc)).reshape(128, E * bc)
        in_maps.append(
            {
                **wimgs,
                "xti": np.ascontiguousarray(xti).astype(np_dt),
                "small": np.ascontiguousarray(small_img).astype(np_dt),
                "bbsm": np.ascontiguousarray(bbsm).astype(np_dt),
            }
        )

    res = run_bass_kernel_spmd(
        nc,
        in_maps,
        core_ids=list(range(N_CORES)),
        trace=PROFILE["trace"],
        tmpdir=PROFILE["tmpdir"],
    )
    LAST_RESULT[0] = res
    return np.concatenate(
        [res.results[c]["out"] for c in range(N_CORES)], axis=0
    )
